# revision 1
# baseline (speedup 1.0000x reference)
"""Trainium2 kernel: depthwise (channel-multiplier-2) 3x3 conv + wing-swap + add.

Reference computes, for input x (B=32, C=256, H=W=56) and weights w (512,1,3,3):
    y[:, 2i], y[:, 2i+1] = conv3x3(x[:, i], w[2i]), conv3x3(x[:, i], w[2i+1])
    out[:, c] = y[:, 2c] + y[:, 2*swap(c)+1]
where swap() exchanges the two 4-channel wings inside each 8-channel butterfly.
Equivalently:  out[:, c] = conv3x3(x[:, c], w[2c]) + conv3x3(x[:, sc], w[2sc+1]),
sc = swap(c).

Strategy (8 NeuronCores, data-parallel over batch, 4 images/core):
  - channels on SBUF partitions, spatial pixels on the free dim
  - host pre-pads W by 1 col each side (zeros) so every tap reads in-bounds
    and every matmul writes a full, contiguous PSUM window
  - per (image, 128-channel half): 9 per-tap 128x128 block-diagonal matmuls
    (fp16 operands, 1 cycle/column, fp32 PSUM accumulate); the wing swap is
    folded into the per-tap weight matrices host-side (2 nonzeros per
    output-channel column)
  - input rows stream in 4 halo'd chunk tiles per unit for fast start and
    fine-grained DMA/compute overlap; ScalarE evacuates PSUM->SBUF
  - measured ~106 us/core on HW (PE roofline ~93 us; DMA ~54 us),
    absmax rel err ~5e-4 vs the fp32 reference
"""

import sys
from contextlib import ExitStack

import numpy as np

for _p in ("/opt/trn_rl_repo",):
    if _p not in sys.path:
        sys.path.insert(0, _p)

import concourse.bass as bass
import concourse.tile as tile
from concourse import bacc, mybir
from concourse.bass_utils import run_bass_kernel_spmd

B, C, H, W = 32, 256, 56, 56
WP = W + 2  # host-padded row width
N_CORES = 8
B_PER = B // N_CORES  # images per core
P = 128               # partitions = channels per half
HALVES = C // P       # 2
RB = 8                # output rows per PSUM block
NRB = H // RB         # 7
NTAPS = 9
BFLY = 8
WING = BFLY // 2

# center tap first: it always writes the full block, so it carries start=True
TAPS = [(0, 0)] + [
    (dh, dw) for dh in (-1, 0, 1) for dw in (-1, 0, 1) if (dh, dw) != (0, 0)
]

_prog_cache = {}


def _swap_local(m: np.ndarray) -> np.ndarray:
    b, r = m // BFLY, m % BFLY
    wng, pos = r // WING, r % WING
    return b * BFLY + (1 - wng) * WING + pos


def _build_weights(w: np.ndarray) -> np.ndarray:
    """Per-tap block-diagonal stationary matrices.

    Returns (P, HALVES*NTAPS*P) f32; wts[k, (h*9+t)*128 + m] is the weight
    from input channel k (partition) to output channel m for tap t of half h.
    """
    w2 = w.reshape(2 * C, NTAPS).astype(np.float32)
    wts = np.zeros((P, HALVES, NTAPS, P), np.float32)
    m = np.arange(P)
    sl = _swap_local(m)
    for h in range(HALVES):
        cg = h * P + m
        sg = h * P + sl
        wts[m, h, :, m] = w2[2 * cg]          # x[c] * w[2c]
        wts[sl, h, :, m] = w2[2 * sg + 1]     # x[sc] * w[2sc+1]
    return np.ascontiguousarray(wts.reshape(P, HALVES * NTAPS * P))


def _build_program(
    loop_iters: int = 1, timing_mode: bool = False, in_dtype: str = "fp16"
) -> bass.Bass:
    # Bacc (not plain Bass): its compile() runs generate_event_semaphores,
    # which splits multi-wait instructions to satisfy the TRN2 1-wait limit
    nc = bacc.Bacc("TRN2", target_bir_lowering=False, debug=False)
    f32 = mybir.dt.float32
    # input dtype trade-off (all run the PE at 1 cycle/column):
    #   f32r: fp32 bits, rel err ~2.2e-4, but 4-byte DMA + slow weight loads
    #   fp16: rel err ~4.1e-4 (11-bit mantissa; |x|<6, |w|<0.5 -> no overflow),
    #         halves input DMA and enables fast (FWL) weight loads
    #   bf16: rel err ~3.6e-3 (8-bit mantissa), same speed as fp16
    in_dt = {
        "f32r": mybir.dt.float32r,
        "fp16": mybir.dt.float16,
        "bf16": mybir.dt.bfloat16,
    }[in_dtype]
    if timing_mode:
        # benchmark-only build: big tensors stay in device DRAM (garbage
        # contents) so wall-time isn't dominated by axon transfers
        x_d = nc.dram_tensor("x_int", [B_PER, C, H, WP], in_dt).ap()
        o_d = nc.dram_tensor("o_int", [B_PER, C, H * W], f32).ap()
        nc.dram_tensor("tiny", [1, 4], f32, kind="ExternalOutput")
    else:
        x_d = nc.dram_tensor("x", [B_PER, C, H, WP], in_dt, kind="ExternalInput").ap()
        o_d = nc.dram_tensor("out", [B_PER, C, H * W], f32, kind="ExternalOutput").ap()
    w_d = nc.dram_tensor("wts", [P, HALVES * NTAPS, P], in_dt, kind="ExternalInput").ap()

    with tile.TileContext(nc) as tc, ExitStack() as ctx:
        wpool = ctx.enter_context(tc.tile_pool(name="wpool", bufs=1))
        xpool = ctx.enter_context(tc.tile_pool(name="xpool", bufs=8))
        opool = ctx.enter_context(tc.tile_pool(name="opool", bufs=4))
        ppool = ctx.enter_context(tc.tile_pool(name="ppool", bufs=4, space="PSUM"))

        # input rows arrive in 4 halo'd chunk tiles per (image, half) — two
        # row-blocks each (+1 row halo both sides) — so the first matmuls
        # start after ~0.5 MB instead of the full 1.7 MB, and unit
        # boundaries pipeline at chunk granularity. Tile tracks deps at
        # tile granularity, hence separate tiles rather than one split DMA.
        CHUNK_LO = [0, 15, 31, 47]          # first input row of each chunk
        CHUNK_HI = [17, 33, 49, 56]         # one past last input row
        CHUNK_ROWS = 18                     # max rows in any chunk
        wts_sb = {}
        for h in range(HALVES):
            wts_sb[h] = wpool.tile(
                [P, NTAPS, P], in_dt, name=f"wt{h}", tag=f"wt{h}"
            )
        # half-0 weights first: first matmul needs only them + chunk 0
        nc.sync.dma_start(out=wts_sb[0], in_=w_d[:, 0:NTAPS, :])
        if loop_iters > 1:
            nc.sync.dma_start(out=wts_sb[1], in_=w_d[:, NTAPS : 2 * NTAPS, :])

        def body():
            for u in range(B_PER * HALVES):
                img, h = divmod(u, HALVES)
                wt = wts_sb[h]
                chunks = []
                for c in range(4):
                    lo, hi = CHUNK_LO[c], CHUNK_HI[c]
                    xt = xpool.tile([P, CHUNK_ROWS, WP], in_dt,
                                    name=f"xt{u}_{c}", tag="xt")
                    nc.sync.dma_start(
                        out=xt[:, 0 : hi - lo, :],
                        in_=x_d[img, h * P : (h + 1) * P, lo:hi, :],
                    )
                    chunks.append(xt)
                if u == 0 and loop_iters == 1:
                    # half-1 weights queued behind unit 0's input chunks
                    nc.sync.dma_start(out=wts_sb[1], in_=w_d[:, NTAPS : 2 * NTAPS, :])
                for rb in range(NRB):
                    r0 = rb * RB
                    ck = min(rb // 2, 3)
                    lo = CHUNK_LO[ck]
                    xt = chunks[ck]
                    ps = ppool.tile([P, RB, W], f32)
                    for i, (dh, dw) in enumerate(TAPS):
                        rs = max(r0, -dh)
                        re = min(r0 + RB, H - dh)
                        t = (dh + 1) * 3 + (dw + 1)
                        nc.tensor.matmul(
                            ps[:, rs - r0 : re - r0, :],
                            wt[:, t, :],
                            xt[:, rs + dh - lo : re + dh - lo, dw + 1 : dw + 1 + W],
                            start=(i == 0),
                            stop=(i == NTAPS - 1),
                        )
                    ot = opool.tile([P, RB * W], f32)
                    nc.scalar.copy(ot, ps.rearrange("p r c -> p (r c)"))
                    nc.sync.dma_start(
                        out=o_d[img, h * P : (h + 1) * P, r0 * W : r0 * W + RB * W],
                        in_=ot,
                    )

        if loop_iters > 1:
            with tc.For_i(0, loop_iters):
                body()
        else:
            body()
    nc.compile()
    return nc


# on-device conv input dtype: "fp16" measured ~14% faster than "f32r" with
# near-identical accuracy (see _build_program comment)
IN_DTYPE = "fp16"

_NP_DT = {"f32r": np.float32, "fp16": np.float16}


def _np_in_dtype():
    if IN_DTYPE == "bf16":
        import ml_dtypes

        return ml_dtypes.bfloat16
    return _NP_DT[IN_DTYPE]


def _get_program() -> bass.Bass:
    key = f"nc_{IN_DTYPE}"
    if key not in _prog_cache:
        _prog_cache[key] = _build_program(in_dtype=IN_DTYPE)
    return _prog_cache[key]


def _run(x: np.ndarray, w: np.ndarray, **run_kwargs):
    """Shard, run on 8 cores, gather. Returns (output, BassKernelResults)."""
    x = np.asarray(x, np.float32).reshape(B, C, H, W)
    xpad = np.zeros((B, C, H, WP), np.float32)
    xpad[:, :, :, 1 : 1 + W] = x
    wts = _build_weights(np.asarray(w, np.float32))
    if IN_DTYPE != "f32r":
        xpad = xpad.astype(_np_in_dtype())
        wts = wts.astype(_np_in_dtype())

    in_maps = [
        {"x": xpad[c * B_PER : (c + 1) * B_PER], "wts": wts.reshape(P, HALVES * NTAPS, P)}
        for c in range(N_CORES)
    ]
    nc = _get_program()
    res = run_bass_kernel_spmd(nc, in_maps, core_ids=list(range(N_CORES)), **run_kwargs)
    out = np.concatenate([res.results[c]["out"] for c in range(N_CORES)], axis=0)
    return out.reshape(B, C, H, W), res


def kernel(x: np.ndarray, w: np.ndarray) -> np.ndarray:
    out, _ = _run(x, w)
    return out



# revision 2
# speedup vs baseline: 1.3399x; 1.3399x over previous
"""Trainium2 kernel: depthwise (channel-multiplier-2) 3x3 conv + wing-swap + add.

Reference, for x (B=32, C=256, H=W=56) and w (512,1,3,3):
    out[:, c] = conv3x3(x[:, c], w[2c]) + conv3x3(x[:, sc], w[2sc+1]),
sc = swap(c) exchanging the two 4-channel wings inside each 8-channel butterfly.

Strategy (8 NeuronCores, data-parallel over batch, 4 images/core):
  Toeplitz-over-rows matmul packing. Per butterfly (8 channels, closed under
  the wing swap), put (channel ci, input row r_in) on the 128 SBUF partitions:
  p = ci*16 + r_in covers a 16-row window producing 14 output rows. The
  128x128 stationary matrix contracts channel + vertical tap simultaneously
  (6 nonzeros per output column: 3 dh taps x 2 convs); the horizontal taps
  dw are 3 PSUM-accumulated matmuls over dw-shifted views of the same moving
  tile. All 4 row-groups x 4 images ride in the matmul free dimension, so a
  butterfly needs just 6 matmuls of 448 columns (2 PSUM-bank halves x 3 dw).

  PE work: 32 bf x 3 dw x 16x56 cols = 86k columns/core (~36 us) vs the
  per-tap block-diagonal approach's 226k (~94 us). Host pre-permutes the
  padded input to the (bf, p, rg*4+img, j) layout so every DMA moves
  >=1.8KB contiguous per partition; output returns as fp16 (~6.4 MB) and is
  cast/un-permuted on the host.
"""

import sys
from contextlib import ExitStack

import numpy as np

for _p in ("/opt/trn_rl_repo",):
    if _p not in sys.path:
        sys.path.insert(0, _p)

import concourse.bass as bass
import concourse.tile as tile
from concourse import bacc, mybir
from concourse.bass_utils import run_bass_kernel_spmd

B, C, H, W = 32, 256, 56, 56
HP, WP = H + 2, W + 2     # zero-padded spatial dims
N_CORES = 8
B_PER = B // N_CORES      # 4 images per core
P = 128                   # SBUF partitions
BFLY, WING = 8, 4
NBF = C // BFLY           # 32 butterflies
RR = 14                   # output rows per row-group
NRG = H // RR             # 4 row-groups
KROWS = RR + 2            # 16-row input window per group
FREE = NRG * B_PER        # 16 free-dim slots: rg*4+img
MOUT = BFLY * RR          # 112 used output columns (padded to 128 for FWL)

_prog_cache = {}


def _swap_local(m):
    b, r = m // BFLY, m % BFLY
    wng, pos = r // WING, r % WING
    return b * BFLY + (1 - wng) * WING + pos


def _build_wst(w: np.ndarray) -> np.ndarray:
    """Stationary matrices wst[k=(ci*16+r_in), bf, dwi, m=(co*14+r_out)].

    m columns [112,128) stay zero so NumWeights==128 keeps FWL enabled.
    """
    w2 = w.reshape(2 * C, 9).astype(np.float32)
    wst = np.zeros((P, NBF, 3, P), np.float32)
    co = np.arange(BFLY)
    sl = np.array([_swap_local(c) for c in co])
    for bf in range(NBF):
        k1 = w2[2 * (bf * BFLY + co)]        # (8, 9) conv-1 kernels
        k2 = w2[2 * (bf * BFLY + sl) + 1]    # (8, 9) conv-2 kernels (swapped wing)
        for c in range(BFLY):
            for r_out in range(RR):
                m = c * RR + r_out
                for dh in (-1, 0, 1):
                    r_in = r_out + dh + 1
                    for dwi in range(3):
                        t = (dh + 1) * 3 + dwi
                        wst[c * KROWS + r_in, bf, dwi, m] += k1[c, t]
                        wst[sl[c] * KROWS + r_in, bf, dwi, m] += k2[c, t]
    return wst


def _make_xprep(xc: np.ndarray) -> np.ndarray:
    """(B_PER, C, HP, WP) padded fp16 -> (NBF, 128, FREE, WP) device layout."""
    win = np.stack([xc[:, :, rg * RR : rg * RR + KROWS, :] for rg in range(NRG)], axis=2)
    win = win.reshape(B_PER, NBF, BFLY, NRG, KROWS, WP)
    win = win.transpose(1, 2, 4, 3, 0, 5)  # bf, ci, r_in, rg, img, j
    return np.ascontiguousarray(win.reshape(NBF, P, FREE, WP))


def _build_program(loop_iters: int = 1, timing_mode: bool = False) -> bass.Bass:
    # Bacc (not plain Bass): its compile() runs generate_event_semaphores,
    # which splits multi-wait instructions to satisfy the TRN2 1-wait limit
    nc = bacc.Bacc("TRN2", target_bir_lowering=False, debug=False)
    f16 = mybir.dt.float16
    f32 = mybir.dt.float32
    if timing_mode:
        # benchmark-only build: big tensors stay in device DRAM (garbage
        # contents) so wall-time isn't dominated by axon transfers
        x_d = nc.dram_tensor("x_int", [NBF, P, FREE, WP], f16).ap()
        o_d = nc.dram_tensor("o_int", [NBF, MOUT, FREE, W], f16).ap()
        nc.dram_tensor("tiny", [1, 4], f32, kind="ExternalOutput")
    else:
        x_d = nc.dram_tensor("x", [NBF, P, FREE, WP], f16, kind="ExternalInput").ap()
        o_d = nc.dram_tensor("out", [NBF, MOUT, FREE, W], f16, kind="ExternalOutput").ap()
    w_d = nc.dram_tensor("wst", [P, NBF, 3, P], f16, kind="ExternalInput").ap()

    with tile.TileContext(nc) as tc, ExitStack() as ctx:
        wpool = ctx.enter_context(tc.tile_pool(name="wpool", bufs=1))
        xpool = ctx.enter_context(tc.tile_pool(name="xpool", bufs=6))
        opool = ctx.enter_context(tc.tile_pool(name="opool", bufs=4))
        ppool = ctx.enter_context(tc.tile_pool(name="ppool", bufs=8, space="PSUM"))

        wt = wpool.tile([P, NBF, 3, P], f16, name="wt", tag="wt")
        nc.sync.dma_start(out=wt, in_=w_d)

        def body():
            for bf in range(NBF):
                xt = xpool.tile([P, FREE, WP], f16, name=f"xt{bf}", tag="xt")
                nc.sync.dma_start(out=xt, in_=x_d[bf])
                ot = opool.tile([MOUT, FREE, W], f16, name=f"ot{bf}", tag="ot")
                for half in range(2):
                    ps = ppool.tile([P, FREE // 2, W], f32)
                    for dwi in range(3):
                        nc.tensor.matmul(
                            ps,
                            wt[:, bf, dwi, :],
                            xt[:, half * 8 : half * 8 + 8, dwi : dwi + W],
                            start=(dwi == 0),
                            stop=(dwi == 2),
                        )
                    nc.scalar.copy(ot[:, half * 8 : half * 8 + 8, :], ps[0:MOUT])
                nc.sync.dma_start(out=o_d[bf], in_=ot)

        if loop_iters > 1:
            with tc.For_i(0, loop_iters):
                body()
        else:
            body()
    nc.compile()
    return nc


def _get_program() -> bass.Bass:
    if "nc" not in _prog_cache:
        _prog_cache["nc"] = _build_program()
    return _prog_cache["nc"]


def _run(x: np.ndarray, w: np.ndarray, **run_kwargs):
    """Shard, run on 8 cores, gather. Returns (output, BassKernelResults)."""
    x = np.asarray(x, np.float32).reshape(B, C, H, W)
    xpad = np.zeros((B, C, HP, WP), np.float16)
    xpad[:, :, 1 : 1 + H, 1 : 1 + W] = x.astype(np.float16)
    wst = _build_wst(np.asarray(w, np.float32)).astype(np.float16)

    in_maps = [
        {"x": _make_xprep(xpad[c * B_PER : (c + 1) * B_PER]), "wst": wst}
        for c in range(N_CORES)
    ]
    nc = _get_program()
    res = run_bass_kernel_spmd(nc, in_maps, core_ids=list(range(N_CORES)), **run_kwargs)
    out = np.empty((B, C, H, W), np.float32)
    for c in range(N_CORES):
        o = np.asarray(res.results[c]["out"], np.float32)
        o = o.reshape(NBF, BFLY, RR, NRG, B_PER, W)
        o = o.transpose(4, 0, 1, 3, 2, 5)  # img, bf, co, rg, r_out, w
        out[c * B_PER : (c + 1) * B_PER] = o.reshape(B_PER, C, H, W)
    return out, res


def kernel(x: np.ndarray, w: np.ndarray) -> np.ndarray:
    out, _ = _run(x, w)
    return out


# revision 3
# speedup vs baseline: 1.8637x; 1.3909x over previous
"""Trainium2 kernel: depthwise (channel-multiplier-2) 3x3 conv + wing-swap + add.

Reference, for x (B=32, C=256, H=W=56) and w (512,1,3,3):
    out[:, c] = conv3x3(x[:, c], w[2c]) + conv3x3(x[:, sc], w[2sc+1]),
sc = swap(c) exchanging the two 4-channel wings inside each 8-channel butterfly.

Strategy (8 NeuronCores, data-parallel over batch, 4 images/core):
  Toeplitz-over-rows matmul packing. Per butterfly (8 channels, closed under
  the wing swap), put (channel ci, input row r_in) on the 128 SBUF partitions:
  p = ci*16 + r_in covers a 16-row window producing 14 output rows. The
  128x128 stationary matrix contracts channel + vertical tap simultaneously
  (6 nonzeros per output column: 3 dh taps x 2 convs); the horizontal taps
  dw are 3 PSUM-accumulated matmuls over dw-shifted views of the same moving
  tile. All 4 row-groups x 4 images ride in the matmul free dimension, so a
  butterfly needs just 6 matmuls of 448 columns (2 PSUM-bank halves x 3 dw).
  PE work: 32 bf x 3 dw x 16x56 cols = 86k columns/core (~37 us) vs the
  per-tap block-diagonal approach's 226k (~94 us).

  DMA: each HWDGE dma_start pays a ~2 us serialized completion stall on its
  ring, so inputs move as 4 chunk DMAs (8 butterflies = 1.9 MB each, host
  pre-permuted so each partition line is 14.8 KB contiguous) on the SP ring,
  while fp16 outputs move as 4 chunk DMAs (1.6 MB) on the Activation ring.
  PSUM evacuation runs on the DVE so it stays off both DMA-issuing queues.
"""

import sys
from contextlib import ExitStack

import numpy as np

for _p in ("/opt/trn_rl_repo",):
    if _p not in sys.path:
        sys.path.insert(0, _p)

import concourse.bass as bass
import concourse.tile as tile
from concourse import bacc, mybir
from concourse.bass_utils import run_bass_kernel_spmd

B, C, H, W = 32, 256, 56, 56
HP, WP = H + 2, W + 2     # zero-padded spatial dims
N_CORES = 8
B_PER = B // N_CORES      # 4 images per core
P = 128                   # SBUF partitions
BFLY, WING = 8, 4
NBF = C // BFLY           # 32 butterflies
RR = 14                   # output rows per row-group
NRG = H // RR             # 4 row-groups
KROWS = RR + 2            # 16-row input window per group
FREE = NRG * B_PER        # 16 free-dim slots: rg*4+img
MOUT = BFLY * RR          # 112 used output columns (padded to 128 for FWL)
CHUNK = 8                 # butterflies per DMA chunk
NCHUNK = NBF // CHUNK     # 4

_prog_cache = {}


def _swap_local(m):
    b, r = m // BFLY, m % BFLY
    wng, pos = r // WING, r % WING
    return b * BFLY + (1 - wng) * WING + pos


def _build_wst(w: np.ndarray) -> np.ndarray:
    """Stationary matrices wst[k=(ci*16+r_in), bf, dwi, m=(co*14+r_out)].

    m columns [112,128) stay zero so NumWeights==128 keeps FWL enabled.
    """
    w2 = w.reshape(2 * C, 9).astype(np.float32)
    wst = np.zeros((P, NBF, 3, P), np.float32)
    co = np.arange(BFLY)
    sl = np.array([_swap_local(c) for c in co])
    for bf in range(NBF):
        k1 = w2[2 * (bf * BFLY + co)]        # (8, 9) conv-1 kernels
        k2 = w2[2 * (bf * BFLY + sl) + 1]    # (8, 9) conv-2 kernels (swapped wing)
        for c in range(BFLY):
            for r_out in range(RR):
                m = c * RR + r_out
                for dh in (-1, 0, 1):
                    r_in = r_out + dh + 1
                    for dwi in range(3):
                        t = (dh + 1) * 3 + dwi
                        wst[c * KROWS + r_in, bf, dwi, m] += k1[c, t]
                        wst[sl[c] * KROWS + r_in, bf, dwi, m] += k2[c, t]
    return wst


def _make_xprep(xc: np.ndarray) -> np.ndarray:
    """(B_PER, C, HP, WP) padded fp16 -> (NCHUNK, 128, CHUNK, FREE, WP)."""
    win = np.stack([xc[:, :, rg * RR : rg * RR + KROWS, :] for rg in range(NRG)], axis=2)
    win = win.reshape(B_PER, NCHUNK, CHUNK, BFLY, NRG, KROWS, WP)
    # -> g, ci, r_in, bfl, rg, img, j
    win = win.transpose(1, 3, 5, 2, 4, 0, 6)
    return np.ascontiguousarray(win.reshape(NCHUNK, P, CHUNK, FREE, WP))


def _build_program(loop_iters: int = 1, timing_mode: bool = False) -> bass.Bass:
    # Bacc (not plain Bass): its compile() runs generate_event_semaphores,
    # which splits multi-wait instructions to satisfy the TRN2 1-wait limit
    nc = bacc.Bacc("TRN2", target_bir_lowering=False, debug=False)
    f16 = mybir.dt.float16
    f32 = mybir.dt.float32
    if timing_mode:
        # benchmark-only build: big tensors stay in device DRAM (garbage
        # contents) so wall-time isn't dominated by axon transfers
        x_d = nc.dram_tensor("x_int", [NCHUNK, P, CHUNK, FREE, WP], f16).ap()
        o_d = nc.dram_tensor("o_int", [NCHUNK, MOUT, CHUNK, FREE, W], f16).ap()
        nc.dram_tensor("tiny", [1, 4], f32, kind="ExternalOutput")
    else:
        x_d = nc.dram_tensor(
            "x", [NCHUNK, P, CHUNK, FREE, WP], f16, kind="ExternalInput"
        ).ap()
        o_d = nc.dram_tensor(
            "out", [NCHUNK, MOUT, CHUNK, FREE, W], f16, kind="ExternalOutput"
        ).ap()
    w_d = nc.dram_tensor("wst", [P, NBF, 3, P], f16, kind="ExternalInput").ap()

    with tile.TileContext(nc) as tc, ExitStack() as ctx:
        wpool = ctx.enter_context(tc.tile_pool(name="wpool", bufs=1))
        xpool = ctx.enter_context(tc.tile_pool(name="xpool", bufs=3))
        opool = ctx.enter_context(tc.tile_pool(name="opool", bufs=3))
        ppool = ctx.enter_context(tc.tile_pool(name="ppool", bufs=8, space="PSUM"))

        wt = wpool.tile([P, NBF, 3, P], f16, name="wt", tag="wt")
        nc.sync.dma_start(out=wt, in_=w_d)

        def body():
            for g in range(NCHUNK):
                xt = xpool.tile([P, CHUNK, FREE, WP], f16, name=f"xt{g}", tag="xt")
                nc.sync.dma_start(out=xt, in_=x_d[g])
                ot = opool.tile([MOUT, CHUNK, FREE, W], f16, name=f"ot{g}", tag="ot")
                for bfl in range(CHUNK):
                    bf = g * CHUNK + bfl
                    for half in range(2):
                        ps = ppool.tile([P, FREE // 2, W], f32)
                        for dwi in range(3):
                            nc.tensor.matmul(
                                ps,
                                wt[:, bf, dwi, :],
                                xt[:, bfl, half * 8 : half * 8 + 8, dwi : dwi + W],
                                start=(dwi == 0),
                                stop=(dwi == 2),
                            )
                        nc.vector.tensor_copy(
                            ot[:, bfl, half * 8 : half * 8 + 8, :], ps[0:MOUT]
                        )
                nc.scalar.dma_start(out=o_d[g], in_=ot)

        if loop_iters > 1:
            with tc.For_i(0, loop_iters):
                body()
        else:
            body()
    nc.compile()
    return nc


def _get_program() -> bass.Bass:
    if "nc" not in _prog_cache:
        _prog_cache["nc"] = _build_program()
    return _prog_cache["nc"]


def _run(x: np.ndarray, w: np.ndarray, **run_kwargs):
    """Shard, run on 8 cores, gather. Returns (output, BassKernelResults)."""
    x = np.asarray(x, np.float32).reshape(B, C, H, W)
    xpad = np.zeros((B, C, HP, WP), np.float16)
    xpad[:, :, 1 : 1 + H, 1 : 1 + W] = x.astype(np.float16)
    wst = _build_wst(np.asarray(w, np.float32)).astype(np.float16)

    in_maps = [
        {"x": _make_xprep(xpad[c * B_PER : (c + 1) * B_PER]), "wst": wst}
        for c in range(N_CORES)
    ]
    nc = _get_program()
    res = run_bass_kernel_spmd(nc, in_maps, core_ids=list(range(N_CORES)), **run_kwargs)
    out = np.empty((B, C, H, W), np.float32)
    for c in range(N_CORES):
        o = np.asarray(res.results[c]["out"], np.float32)
        o = o.reshape(NCHUNK, BFLY, RR, CHUNK, NRG, B_PER, W)
        o = o.transpose(5, 0, 3, 1, 4, 2, 6)  # img, g, bfl, co, rg, r_out, w
        out[c * B_PER : (c + 1) * B_PER] = o.reshape(B_PER, C, H, W)
    return out, res


def kernel(x: np.ndarray, w: np.ndarray) -> np.ndarray:
    out, _ = _run(x, w)
    return out
